# revision 14
# baseline (speedup 1.0000x reference)
"""Trainium2 Bass kernel for nn_MlroleNode_64716567216639 (GAT message passing).

Only row 0 of the NxN GATv2 attention feeds the output, so the kernel computes
just that row. All matmuls run in bf16 (fp32 matmul on TRN2 is a 2-pass
LOW/HIGH split at ~4x the cost); accumulation stays fp32 in PSUM. Inputs are
packed host-side into 3 DRAM buffers on two DMA rings (gpsimd SWDGE + sync
HWDGE) so completions overlap. The role-type trans+merge composition is
linear, so K_t = W_merge_R @ W_trans_t and its bias are folded on the host,
shortening the serial merge chain. Bias+LeakyReLU is fused into single ACT
ops (Prelu, same table set as Exp); sigmoid is computed via tanh (same set).
h1 (node 0) is written into a reserved bf16 column adjacent to the ambiguous
columns so the pairwise job is two clean 512-col chunks per block, and the
amb-only chunks are emitted before the h1 write so they run during the chain.

Layout: features on partitions, nodes on the free axis. GAT row-0 work is
replicated on all 8 cores; the final MLP is sharded 128 nodes per core.
"""
import numpy as np

H = 64
N_AMB = 1023
HEADS = 4
HID = 64
RT = 4
APT = 3
SLOPE = 0.2
NCORES = 8
SHARD = 128

# packP (bf16, gpsimd ring): prologue weights + small operands
P_HID = 0
P_WSELF = 1
P_WML = 65
P_KT = 129      # 256 cols: (W_merge_R @ W_trans_t).T per role
P_TA = 385      # 12 cols
P_CB = 397      # 4 cols: W_merge_R @ b_trans_t + b_merge
P_BSC = 401     # 1 col
P_BD2H = 402    # 1 col: 0.5*bd2 in rows 0..3
P_COLS = 403

# packB (bf16, sync ring): GAT weights + ambiguous nodes
B_WL = 0        # 256 cols (two 128-col head-pair blocks)
B_WR = 256      # 256 cols
B_WD0B = 512    # 64 cols
B_H1 = 576      # 1 col: h1 slot, written on device
B_AMB = 577     # 1023 cols
B_COLS = 1600

# packC (bf16, sync ring)
C_WEXP = 0      # 128
C_FOLD = 128    # 64
C_WD1 = 192     # 128 (rows 0..64)
C_WD2 = 320     # 4
C_WD0A = 324    # 64 (rows 0..64)
C_MLP = 388     # 128 (rows 0..64, row 64 = ones)
C_KC = 516      # 64 cols: 0.25 * [Wd0b.T; Wd0b.T] (fold matrix pre-applied)
C_COLS = 580

_compiled = None


def _build():
    import concourse.tile as tile
    from concourse import bacc, mybir

    f32 = mybir.dt.float32
    bf16 = mybir.dt.bfloat16
    AF = mybir.ActivationFunctionType
    ALU = mybir.AluOpType
    AX = mybir.AxisListType

    nc = bacc.Bacc("TRN2", target_bir_lowering=False, debug=False,
                   enable_asserts=False, num_devices=NCORES)

    pp_d = nc.dram_tensor("packP", [H, P_COLS], bf16, kind="ExternalInput").ap()
    pb_d = nc.dram_tensor("packB", [H, B_COLS], bf16, kind="ExternalInput").ap()
    pc_d = nc.dram_tensor("packC", [128, C_COLS], bf16, kind="ExternalInput").ap()
    outT_d = nc.dram_tensor("outT", [RT, SHARD], f32, kind="ExternalOutput").ap()

    with tile.TileContext(nc) as tc:
        with tc.tile_pool(name="wp", bufs=1) as wp, \
             tc.tile_pool(name="sb", bufs=1) as sb, \
             tc.tile_pool(name="ps", bufs=1, space="PSUM") as ps:

            # ---- ACT table warm (Exp/Prelu/Tanh share exp_and_others) ----
            warm = wp.tile([1, 4], f32, tag="warm")
            nc.vector.memset(warm[:], 0.0)
            warm_o = wp.tile([1, 4], f32, tag="warmo")
            nc.scalar.activation(warm_o[0:1, 0:1], warm[0:1, 0:1], AF.Exp)

            # ---- input loads on two independent DMA rings: the small
            # chain-critical pack on the faster sync HWDGE ring, the big
            # GAT pack on the gpsimd SWDGE ring in parallel ----
            pP = wp.tile([H, P_COLS], bf16, tag="pP")
            nc.sync.dma_start(pP[:], pp_d[:])
            pB = wp.tile([H, B_COLS], bf16, tag="pB")
            nc.gpsimd.dma_start(pB[:], pb_d[:])
            pC = wp.tile([128, C_COLS], bf16, tag="pC")
            nc.sync.dma_start(pC[:], pc_d[:])

            y0_aug = sb.tile([65, SHARD], bf16, tag="y0a")
            nc.vector.memset(y0_aug[64:65, :], 1.0)

            WlT = [pB[0:H, B_WL + 128 * b:B_WL + 128 * (b + 1)] for b in (0, 1)]
            WrT = [pB[0:H, B_WR + 128 * b:B_WR + 128 * (b + 1)] for b in (0, 1)]
            Wexp = pC[0:128, C_WEXP:C_WEXP + 128]
            # rhs halves: h0 includes the h1 column (B_H1) + amb cols 0..510
            rhs_h = [pB[0:H, B_H1:B_H1 + 512], pB[0:H, B_H1 + 512:B_H1 + 1024]]

            # fp32 shadow of the small bias columns (ACT/DVE bias APs)
            bias_f = sb.tile([H, 6], f32, tag="biasf")
            nc.vector.tensor_copy(bias_f[:], pP[0:H, P_CB:P_CB + 6])
            cb_f = bias_f[:, 0:4]
            bsc_f = bias_f[:, 4:5]
            bd2h_f = bias_f[0:RT, 5:6]

            # ---- prologue: h1_0 and the K-folded role contributions ----
            h1_ps = ps.tile([H, 1], f32, tag="sp", bufs=2)
            nc.tensor.matmul(h1_ps[:], pP[0:H, P_WSELF:P_WSELF + H],
                             pP[0:H, P_HID:P_HID + 1], start=True, stop=True)
            h1 = sb.tile([H, 1], bf16, tag="h1", bufs=2)
            nc.vector.tensor_scalar_add(h1[:], h1_ps[:], bsc_f)

            tsum = sb.tile([H, RT], f32, tag="tsum")
            nc.vector.reduce_sum(
                tsum[:], pP[0:H, P_TA:P_TA + RT * APT].rearrange(
                    "p (t a) -> p t a", a=APT), axis=AX.X)
            tmean = sb.tile([H, RT], bf16, tag="tmean")
            nc.vector.tensor_scalar_mul(tmean[:], tsum[:], 1.0 / APT)
            C_ps = ps.tile([H, RT], f32, tag="sp", bufs=2)
            for t in range(RT):
                nc.tensor.matmul(C_ps[:, t:t + 1],
                                 pP[0:H, P_KT + H * t:P_KT + H * (t + 1)],
                                 tmean[:, t:t + 1], start=True, stop=True)
            C_sb = sb.tile([H, RT], f32, tag="C")
            nc.vector.tensor_tensor(C_sb[:], C_ps[:], cb_f, op=ALU.add)

            # ---- serial merge chain: h1 <- lrelu(WmL@h1 + C_t) ----
            for t in range(RT):
                hp = ps.tile([H, 1], f32, tag="sp", bufs=2)
                nc.tensor.matmul(hp[:], pP[0:H, P_WML:P_WML + H], h1[:],
                                 start=True, stop=True)
                if t < RT - 1:
                    h1n = sb.tile([H, 1], bf16, tag="h1", bufs=2)
                    nc.scalar.activation(h1n[:], hp[:], AF.Prelu,
                                         bias=C_sb[:, t:t + 1], alpha=SLOPE)
                    h1 = h1n
                else:
                    nc.scalar.activation(pB[0:H, B_H1:B_H1 + 1], hp[:], AF.Prelu,
                                         bias=C_sb[:, t:t + 1], alpha=SLOPE)

            h1col = pB[0:H, B_H1:B_H1 + 1]

            # gl cols 1..1023 read only ambiguous columns (no h1 dep): emit
            # them in ~256-col chunks, block0 first, so the scheduler
            # backfills PE gaps during the chain without long matmuls
            # delaying the chain's own matmuls. Only the col-0 (h1) sliver
            # runs after the chain.
            gl_ps = {0: ps.tile([128, 1024], f32, tag="big", bufs=2, name="gl0"),
                     1: ps.tile([128, 1024], f32, tag="big", bufs=2, name="gl1")}
            for b in (0, 1):
                for lo, hi in ((1, 256), (256, 512), (512, 768), (768, 1024)):
                    nc.tensor.matmul(gl_ps[b][:, lo:hi], WlT[b],
                                     pB[0:H, B_H1 + lo:B_H1 + hi],
                                     start=True, stop=True)

            # attention query side: gr0 column per block + the h1 gl sliver
            gr0c = []
            for b in (0, 1):
                g_ps = ps.tile([128, 1], f32, tag="sp", bufs=2)
                nc.tensor.matmul(g_ps[:], WrT[b], h1col, start=True, stop=True)
                g_sb = sb.tile([128, 1], f32, tag="gr0", bufs=2)
                nc.vector.tensor_copy(g_sb[:], g_ps[:])
                gr0c.append(g_sb)
            for b in (0, 1):
                nc.tensor.matmul(gl_ps[b][:, 0:1], WlT[b],
                                 pB[0:H, B_H1:B_H1 + 1], start=True, stop=True)

            # ---- t = lrelu(gl + gr0), one full-width ACT op per block ----
            t_sb = []
            for b in (0, 1):
                t_t = sb.tile([128, 1024], bf16, tag="t", bufs=2)
                nc.scalar.activation(t_t[:], gl_ps[b][:], AF.Prelu,
                                     bias=gr0c[b][:], alpha=SLOPE)
                t_sb.append(t_t)

            # ---- e = Wexp.T @ t; pexp = exp(e) full-width with accum;
            # weighted value sum per 512-half. Emission order matters for the
            # psum slot reuse (block1 gr reuses block0's slots). ----
            pexp = []
            ssum = []
            rs = []
            gr_ps = {}
            att_u = {}

            def e_exp(b):
                e = ps.tile([128, 1024], f32, tag="big", bufs=2)
                for h in (1, 0):
                    sl = slice(512 * h, 512 * (h + 1))
                    nc.tensor.matmul(e[:, sl], Wexp, t_sb[b][:, sl],
                                     start=True, stop=True)
                p = sb.tile([128, 1024], f32, tag="pexp", bufs=2)
                s = sb.tile([128, 1], f32, tag="s", bufs=2)
                nc.scalar.activation(p[:], e[:], AF.Exp, bias=0.0, accum_out=s[:])
                pexp.append(p)
                ssum.append(s)

            def recip(b):
                r = sb.tile([128, 1], f32, tag="rs", bufs=2)
                nc.vector.reciprocal(r[:], ssum[b][:])
                rs.append(r)

            def gr_mm(b, h):
                g = ps.tile([128, 512], f32, tag="gr", bufs=2)
                nc.tensor.matmul(g[:], WrT[b], rhs_h[h], start=True, stop=True)
                gr_ps[(b, h)] = g

            def wsum(b, h):
                sl = slice(512 * h, 512 * (h + 1))
                scr = sb.tile([128, 512], bf16, tag="scr", bufs=2)
                a = sb.tile([128, 1], f32, tag="au", bufs=4)
                nc.vector.scalar_tensor_tensor(
                    out=scr[:], in0=pexp[b][:, sl], scalar=1.0,
                    in1=gr_ps[(b, h)][:], op0=ALU.mult, op1=ALU.mult,
                    accum_out=a[:])
                att_u[(b, h)] = a

            # c0 = Wd0b @ mean_heads(attn) folded into one accumulated
            # matmul pair: lhsT = 0.25*[Wd0b.T; Wd0b.T] (host-precomputed)
            c0_ps = ps.tile([H, 1], f32, tag="sp", bufs=2)

            def combine_fold(b):
                att_n = sb.tile([128, 1], bf16, tag="an", bufs=2)
                nc.vector.tensor_scalar(att_n[:], att_u[(b, 0)][:],
                                        att_u[(b, 1)][:], rs[b][:],
                                        op0=ALU.add, op1=ALU.mult)
                nc.tensor.matmul(c0_ps[:], pC[0:128, C_KC:C_KC + H], att_n[:],
                                 start=(b == 0), stop=(b == 1))

            e_exp(0)
            gr_mm(0, 0)
            gr_mm(0, 1)
            wsum(0, 0)
            wsum(0, 1)
            recip(0)
            e_exp(1)
            gr_mm(1, 0)
            gr_mm(1, 1)
            combine_fold(0)
            wsum(1, 0)
            wsum(1, 1)
            recip(1)
            combine_fold(1)

            # ---- final MLP on this core's 128-node shard ----
            c0col = sb.tile([H, 1], f32, tag="c0")
            nc.vector.tensor_copy(c0col[:], c0_ps[:])
            y0_ps = ps.tile([H, SHARD], f32, tag="gr", bufs=2)
            nc.tensor.matmul(y0_ps[:], pC[0:65, C_WD0A:C_WD0A + H],
                             pC[0:65, C_MLP:C_MLP + SHARD], start=True, stop=True)
            nc.scalar.activation(y0_aug[0:H, :], y0_ps[:], AF.Prelu,
                                 bias=c0col[:], alpha=SLOPE)
            y1_ps = ps.tile([128, SHARD], f32, tag="gr", bufs=2)
            nc.tensor.matmul(y1_ps[:], pC[0:65, C_WD1:C_WD1 + 128], y0_aug[:],
                             start=True, stop=True)
            y1 = sb.tile([128, SHARD], bf16, tag="y1")
            nc.scalar.activation(y1[:], y1_ps[:], AF.Prelu, bias=0.0, alpha=SLOPE)
            o_ps = ps.tile([RT, SHARD], f32, tag="sp", bufs=2)
            nc.tensor.matmul(o_ps[:], pC[0:128, C_WD2:C_WD2 + RT], y1[:],
                             start=True, stop=True)
            # sigmoid(z) = 0.5 + 0.5*tanh(0.5*z + 0.5*bd2); tanh shares the
            # already-loaded exp table set
            th = sb.tile([RT, SHARD], f32, tag="th")
            nc.scalar.activation(th[:], o_ps[:], AF.Tanh, bias=bd2h_f, scale=0.5)
            o_sb = sb.tile([RT, SHARD], f32, tag="o")
            nc.vector.tensor_scalar(o_sb[:], th[:], 0.5, 0.5,
                                    op0=ALU.mult, op1=ALU.add)
            nc.sync.dma_start(outT_d[:], o_sb[:])

    nc.compile()
    return nc


def _prep_inputs(inputs):
    import ml_dtypes
    f32 = np.float32
    bf = ml_dtypes.bfloat16

    hidden = np.asarray(inputs["hidden"], f32)
    ambiguous = np.asarray(inputs["ambiguous"], f32)
    type_agents = np.asarray(inputs["type_agents"], f32)
    W_self = np.asarray(inputs["W_self"], f32)
    b_self = np.asarray(inputs["b_self"], f32)
    W_merge = np.asarray(inputs["W_merge"], f32)
    b_merge = np.asarray(inputs["b_merge"], f32)
    W_trans = np.asarray(inputs["W_trans"], f32)
    b_trans = np.asarray(inputs["b_trans"], f32)
    W_l = np.asarray(inputs["W_l"], f32)
    W_r = np.asarray(inputs["W_r"], f32)
    w_attn = np.asarray(inputs["w_attn"], f32)
    Wd0 = np.asarray(inputs["Wd0"], f32)
    bd0 = np.asarray(inputs["bd0"], f32)
    Wd1 = np.asarray(inputs["Wd1"], f32)
    bd1 = np.asarray(inputs["bd1"], f32)
    Wd2 = np.asarray(inputs["Wd2"], f32)
    bd2 = np.asarray(inputs["bd2"], f32)

    WmL, WmR = W_merge[:, :H], W_merge[:, H:]

    packP = np.zeros((H, P_COLS), f32)
    packP[:, P_HID] = hidden[0]
    packP[:, P_WSELF:P_WSELF + H] = W_self.T
    packP[:, P_WML:P_WML + H] = WmL.T
    # K_t = WmR @ W_trans_t folded on host; lhsT layout needs K_t.T
    packP[:, P_KT:P_KT + RT * H] = np.concatenate(
        [(WmR @ W_trans[t]).T for t in range(RT)], axis=1)
    packP[:, P_TA:P_TA + RT * APT] = type_agents.reshape(RT * APT, H).T
    packP[:, P_CB:P_CB + RT] = np.stack(
        [WmR @ b_trans[t] + b_merge for t in range(RT)], axis=1)
    packP[:, P_BSC] = b_self
    packP[0:RT, P_BD2H] = 0.5 * bd2

    packB = np.zeros((H, B_COLS), f32)
    packB[:, B_WL:B_WL + 2 * 128] = W_l.T
    packB[:, B_WR:B_WR + 2 * 128] = W_r.T
    packB[:, B_WD0B:B_WD0B + H] = Wd0[:, H:].T
    packB[:, B_AMB:B_AMB + N_AMB] = ambiguous.T

    Wexp = np.zeros((128, 128), f32)
    for hh in range(2):
        Wexp[hh * 64:(hh + 1) * 64, hh * 64:(hh + 1) * 64] = w_attn[:, None]
    fold = np.zeros((128, H), f32)
    fold[np.arange(128), np.arange(128) % H] = 0.25

    packC = np.zeros((128, C_COLS), f32)
    packC[:, C_WEXP:C_WEXP + 128] = Wexp
    packC[:, C_FOLD:C_FOLD + H] = fold
    packC[0:65, C_WD1:C_WD1 + 128] = np.vstack([Wd1.T, bd1[None, :]])
    packC[:, C_WD2:C_WD2 + RT] = Wd2.T
    packC[0:65, C_WD0A:C_WD0A + H] = np.vstack([Wd0[:, :H].T, bd0[None, :]])
    packC[64, C_MLP:C_MLP + SHARD] = 1.0
    packC[:, C_KC:C_KC + H] = 0.25 * np.vstack([Wd0[:, H:].T, Wd0[:, H:].T])

    amb_pad = np.zeros((H, NCORES * SHARD), f32)
    amb_pad[:, :N_AMB] = ambiguous.T
    packP16 = packP.astype(bf)
    packB16 = packB.astype(bf)
    in_maps = []
    for cidx in range(NCORES):
        pc = packC.copy()
        pc[0:H, C_MLP:C_MLP + SHARD] = amb_pad[:, cidx * SHARD:(cidx + 1) * SHARD]
        in_maps.append({
            "packP": packP16,
            "packB": packB16,
            "packC": pc.astype(bf),
        })
    return in_maps


def kernel(**inputs) -> np.ndarray:
    global _compiled
    if _compiled is None:
        _compiled = _build()
    nc = _compiled
    from concourse import bass_utils

    in_maps = _prep_inputs(inputs)
    res = bass_utils.run_bass_kernel_spmd(nc, in_maps, core_ids=list(range(NCORES)))
    out = np.empty((N_AMB, RT), np.float32)
    for cidx in range(NCORES):
        lo = cidx * SHARD
        hi = min(lo + SHARD, N_AMB)
        out[lo:hi, :] = res.results[cidx]["outT"][:, :hi - lo].T
    return out
